# revision 47
# baseline (speedup 1.0000x reference)
"""Trainium2 Bass kernel for nn_BulkSpaceGenerator.

Computes, for boundary_tokens x (B, N, D), W1 (D, K*D), b1 (K*D,):
    bulk   = x @ W1 + b1                    -> (B, N, K, D)
    inc    = |delta_n bulk| * (ads/z_k)     (delta along sequence, first row = bulk[0])
    out    = cumsum_n(inc).mean(k)          -> (B, N, D)

Key algebraic restructuring:
  - mean over k commutes with the cumsum, so out = cumsum_n(mean_k(warp_k*|delta|)).
  - delta_n bulk = (delta_n x) @ W1 (bias cancels for n>0), so we matmul the
    *differenced* input once instead of materializing bulk.
  - warp_k/K is positive, so it folds into W1's columns: |dx @ (W1*s_k)| = s_k|dx @ W1|.

The big matmul runs in fp8-e4m3 with perf_mode=DoubleRow (2 fp8 weights per PE
cell, contraction 256 per instruction) for ~1.9x TensorE throughput over bf16.
Weights carry a global x64 scale (keeps fp8 values out of the subnormal
range); since everything after the |.| is linear, the 1/64 descale folds into
the final PSUM->SBUF output copy.

Sharding: 8 shards over (B=2) x (4 sequence chunks of 1024 tokens). Each core
computes its chunk's per-token increments m = sum_k |dxT.T @ W1s|_k and the
local cumsum on-device; the host adds the (tiny) cross-chunk prefix offsets.

Device layout per core (tokens on PSUM partitions, so the local cumsum is a
matmul with a triangular ones matrix and output rows DMA out contiguously):
  dxt  (128, 8, 8, 128) fp8  [p, tb, cb, t] = dx_chunk[tb*128+t, cb*128+p]
  w<k> (128, 8, 1024) fp8    [p, cb, d]     = (W1 * 64*s_k)[cb*128+p, k*1024+d]
  out  (1024, 1024) bf16     local cumsum of m over the chunk (scaled 1/64)

The k-sum accumulates in bf16 (ACT abs-evacuates PSUM -> bf16, DVE adds at 2x
rate); the cumsum is tri@acc[tb] + ones@S_bf[tb] per 128-token block. S_bf is
a per-partition bf16 running sum of whole blocks -- the ones-matmul reduces it
over partitions in f32 PSUM, so each stored entry stays ~128x below the true
prefix and the bf16 chain error is negligible. Emission lags its block's
evacuation by one block so the cumsum matmuls never stall the PE.
"""

import os
import sys
import types
import numpy as np
import ml_dtypes

D = 1024
K = 10
B = 2
N = 4096
ADS_RADIUS = 1.0
NCORES = 8
CHUNK = 1024            # tokens per core
KD = K * D
CB = 8                  # contraction blocks (D / 128)
TBLK = 8                # token blocks per chunk (CHUNK / 128)
# kd columns per weight group: one k slice per group
GROUP_COLS = [D] * K
GROUP_LO = [sum(GROUP_COLS[:g]) for g in range(len(GROUP_COLS))]
NGROUPS = len(GROUP_COLS)

BF16 = ml_dtypes.bfloat16
FP8 = ml_dtypes.float8_e4m3   # TRN FP8_EXP4: max normal +-240
FP8_MAX = 240.0
WSCALE = 64.0                 # global fp8 weight scale, descaled at output

_CACHE = {}


def _install_ntff_hook():
    """Best-effort: register the axon NTFF profiling hook so BASS_TRACE=1 works.

    The agent image's antenv package lacks axon_hooks; inject a shim module and
    wire it to the ctypes-based hook from trn_agent_boot. Harmless if anything
    is missing -- tracing is simply skipped.
    """
    try:
        import antenv
        if "antenv.axon_hooks" in sys.modules:
            return
        hooks = []
        mod = types.ModuleType("antenv.axon_hooks")
        mod.set_axon_ntff_profile_hook = hooks.append
        mod.get_axon_ntff_profile_hook = lambda: (hooks[-1] if hooks else None)
        sys.modules["antenv.axon_hooks"] = mod
        antenv.axon_hooks = mod
        from trn_agent_boot.trn_boot import _ntff_profile_via_ctypes
        h = _ntff_profile_via_ctypes("/opt/axon/libaxon_pjrt.so")
        if h is not None:
            mod.set_axon_ntff_profile_hook(h)
    except Exception:
        pass


def _build():
    from concourse import bacc
    import concourse.mybir as mybir
    import concourse.tile as tile

    fp32 = mybir.dt.float32
    bf16 = mybir.dt.bfloat16
    fp8 = mybir.dt.float8e4
    ADD = mybir.AluOpType.add
    ABS = mybir.ActivationFunctionType.Abs
    DR = mybir.MatmulPerfMode.DoubleRow

    nc = bacc.Bacc()
    dxt = nc.declare_dram_parameter("dxt", [128, TBLK, CB, 128], fp8, isOutput=False)
    wg = [
        nc.declare_dram_parameter(f"w{g}", [128, CB, GROUP_COLS[g]], fp8, isOutput=False)
        for g in range(NGROUPS)
    ]
    tri = nc.declare_dram_parameter("tri", [128, 128], bf16, isOutput=False)
    ones = nc.declare_dram_parameter("ones", [128, 128], bf16, isOutput=False)
    out = nc.declare_dram_parameter("out", [CHUNK, D], bf16, isOutput=True)

    with tile.TileContext(nc) as tc:
        with (
            tc.tile_pool(name="const", bufs=1) as cpool,
            tc.tile_pool(name="dx", bufs=1) as dxpool,
            tc.tile_pool(name="w", bufs=2) as wpool,
            tc.tile_pool(name="acc", bufs=1) as accpool,
            tc.tile_pool(name="tmp", bufs=3) as tpool,
            tc.tile_pool(name="outs", bufs=3) as opool,
            tc.tile_pool(name="ps", bufs=2, space="PSUM") as ppool,
            tc.tile_pool(name="pc", bufs=4, space="PSUM") as pcpool,
        ):
            tri_sb = cpool.tile([128, 128], bf16, tag="tri")
            ones_sb = cpool.tile([128, 128], bf16, tag="ones")
            dx_sb = dxpool.tile([128, TBLK, CB, 128], fp8, tag="dxt")

            # small constants ride the ACT HWDGE ring so they never queue
            # ahead of the weight/dx stream on the sync ring
            nc.scalar.dma_start(out=tri_sb[:], in_=tri[:])
            nc.scalar.dma_start(out=ones_sb[:], in_=ones[:])

            acc = accpool.tile([128, TBLK, D], bf16, tag="acc")
            # S_bf[tb] = per-partition bf16 running sum of acc[0..tb-1]; the
            # ones-matmul reduces over partitions, so each entry stays ~128x
            # below the true prefix and the bf16 chain error is negligible
            S_bf = accpool.tile([128, TBLK, D], bf16, tag="Sbf")

            def emit(tb):
                # out rows of block tb = tri @ acc[tb] + ones @ S_bf[tb]
                ot = opool.tile([128, D], bf16, tag="ot", name="ot")
                for h in range(2):
                    hs = slice(h * 512, (h + 1) * 512)
                    pc = pcpool.tile([128, 512], fp32, tag="pc", name="pc")
                    nc.tensor.matmul(
                        pc[:], lhsT=tri_sb[:], rhs=acc[:, tb, hs],
                        start=True, stop=(tb == 0),
                    )
                    if tb > 0:
                        nc.tensor.matmul(
                            pc[:], lhsT=ones_sb[:], rhs=S_bf[:, tb, hs],
                            start=False, stop=True,
                        )
                    # descale + PSUM->SBUF on two engines in parallel; each
                    # half DMAs out as it lands, on the (long idle) sync ring
                    if h == 0:
                        nc.scalar.mul(ot[:, hs], pc[:], 1.0 / WSCALE)
                    else:
                        nc.vector.tensor_scalar_mul(ot[:, hs], pc[:], 1.0 / WSCALE)
                    nc.sync.dma_start(
                        out=out[tb * 128:(tb + 1) * 128, hs], in_=ot[:, hs]
                    )

            def dma_dx(tb):
                nc.sync.dma_start(out=dx_sb[:, tb, :, :], in_=dxt[:, tb, :, :])

            for g in range(NGROUPS):
                gcols = GROUP_COLS[g]
                jt = gcols // 512
                wt = wpool.tile([128, CB, gcols], fp8, tag="wt", name="wt")

                def dma_w(p, g=g, wt=wt):
                    nc.sync.dma_start(
                        out=wt[:, 2 * p:2 * p + 2, :], in_=wg[g][:, 2 * p:2 * p + 2, :]
                    )

                if g == 0:
                    # the first matmul needs only dx slab 0 + weight pair 0;
                    # land those first. Pairs 2/3 ride the scalar HWDGE ring
                    # in parallel so block 0's full contraction arrives in
                    # time, while the dx slabs stream on the sync ring.
                    dma_dx(0)
                    dma_w(0)
                    dma_w(1)
                    nc.scalar.dma_start(out=wt[:, 4:6, :], in_=wg[0][:, 4:6, :])
                    nc.scalar.dma_start(out=wt[:, 6:8, :], in_=wg[0][:, 6:8, :])
                    for tb in range(1, TBLK):
                        dma_dx(tb)
                else:
                    for p in range(CB // 2):
                        dma_w(p)

                last = g == NGROUPS - 1
                # PSUM accumulation order is free; for group 0 follow the
                # order the weight pairs actually arrive on the two rings
                # (pair 1 queues behind the dx slabs on the sync ring)
                cbp_order = [0, 2, 3, 1] if g == 0 else [0, 1, 2, 3]
                base = GROUP_LO[g] // 512          # first 512-col kd tile
                for tb in range(TBLK):
                    ps = ppool.tile([128, gcols], fp32, tag="ps", name="ps")
                    for ci, cbp in enumerate(cbp_order):
                        lhsT = dx_sb[:, tb, 2 * cbp:2 * cbp + 2, :]
                        for j in range(jt):
                            nc.tensor.matmul(
                                ps[:, j * 512:(j + 1) * 512],
                                lhsT=lhsT,
                                rhs=wt[:, 2 * cbp:2 * cbp + 2, j * 512:(j + 1) * 512],
                                start=(ci == 0),
                                stop=(ci == CB // 2 - 1),
                                perf_mode=DR,
                            )
                    # evacuate |psum| and accumulate into acc[tb]; kd tile
                    # base+j lands at acc column ((base+j)%2)*512, merging
                    # tile pairs whose destinations are contiguous
                    if last and tb == TBLK - 1:
                        # split the final evacuation into halves so the last
                        # block's emission matmuls start on half 0 while half
                        # 1 is still coming out of PSUM (shortens the tail)
                        tmp = tpool.tile([128, gcols], bf16, tag="tmp", name="tmp")
                        for h in range(2):
                            hs = slice(h * 512, (h + 1) * 512)
                            nc.scalar.activation(tmp[:, hs], ps[:, hs], ABS)
                            nc.vector.tensor_tensor(
                                acc[:, tb, hs], acc[:, tb, hs], tmp[:, hs], ADD,
                            )
                    else:
                        # kd tiles 0/1 are the first touch of their acc half:
                        # ACT writes |psum| straight in, no add needed
                        tmp = None
                        if base + jt > 2:
                            tmp = tpool.tile([128, gcols], bf16, tag="tmp", name="tmp")
                            nc.scalar.activation(tmp[:], ps[:], ABS)
                        j = 0
                        while j < jt:
                            w = 2 if ((base + j) % 2 == 0 and j + 1 < jt) else 1
                            doff = ((base + j) % 2) * 512
                            dest = acc[:, tb, doff:doff + 512 * w]
                            if base + j < 2:
                                nc.scalar.activation(
                                    dest, ps[:, j * 512:(j + w) * 512], ABS
                                )
                            else:
                                nc.vector.tensor_tensor(
                                    dest, dest, tmp[:, j * 512:(j + w) * 512], ADD,
                                )
                            j += w
                    if last:
                        if tb >= 1:
                            if tb == 1:
                                nc.vector.tensor_copy(S_bf[:, 1, :], acc[:, 0, :])
                            else:
                                nc.vector.tensor_tensor(
                                    S_bf[:, tb, :], S_bf[:, tb - 1, :],
                                    acc[:, tb - 1, :], ADD,
                                )
                            # emission lags one block so its matmuls never
                            # wait on this block's evacuation chain
                            emit(tb - 1)
            emit(TBLK - 1)

    nc.compile()
    return nc


def _get_nc():
    if "nc" not in _CACHE:
        _CACHE["nc"] = _build()
    return _CACHE["nc"]


def kernel(boundary_tokens: np.ndarray, W1: np.ndarray, b1: np.ndarray) -> np.ndarray:
    from concourse.bass_utils import run_bass_kernel_spmd

    _install_ntff_hook()

    x = np.asarray(boundary_tokens, dtype=np.float32)
    W1 = np.asarray(W1, dtype=np.float32)
    b1 = np.asarray(b1, dtype=np.float32)
    assert x.shape == (B, N, D) and W1.shape == (D, KD)

    # host prep: difference along the sequence, fold warp/K scaling into W1
    dx = np.empty_like(x)
    dx[:, 0] = x[:, 0]
    dx[:, 1:] = x[:, 1:] - x[:, :-1]

    scale = (1.0 / (np.arange(K, dtype=np.float32) + 1.0))  # warp_k / K = 1/(k+1)
    W1s = (W1.reshape(D, K, D) * (WSCALE * scale)[None, :, None]).reshape(D, KD)
    W1q = np.clip(W1s, -FP8_MAX, FP8_MAX).astype(FP8)
    # [p, cb, col] per column group, each contiguous so weight DMAs get >=2KB rows
    w_r = W1q.reshape(CB, 128, KD)
    w_groups = [
        np.ascontiguousarray(
            w_r[:, :, GROUP_LO[g]:GROUP_LO[g] + GROUP_COLS[g]].transpose(1, 0, 2)
        )
        for g in range(NGROUPS)
    ]

    idx = np.arange(128)
    tri = (idx[:, None] <= idx[None, :]).astype(BF16)   # tri[s,t]=1 iff s<=t
    ones = np.ones((128, 128), dtype=BF16)

    chunks_per_b = N // CHUNK
    in_maps = []
    for core in range(NCORES):
        b, c = divmod(core, chunks_per_b)
        dxc = dx[b, c * CHUNK:(c + 1) * CHUNK]          # (CHUNK, D)
        dxq = np.clip(dxc, -FP8_MAX, FP8_MAX).astype(FP8)
        # [p, tb, cb, t]: per token-block slab, contiguous 1KB rows
        dxt = np.ascontiguousarray(
            dxq.T.reshape(CB, 128, TBLK, 128).transpose(1, 2, 0, 3)
        )
        im = {"dxt": dxt, "tri": tri, "ones": ones}
        for g in range(NGROUPS):
            im[f"w{g}"] = w_groups[g]
        in_maps.append(im)

    res = run_bass_kernel_spmd(
        _get_nc(), in_maps, list(range(NCORES)),
        trace=bool(os.environ.get("BASS_TRACE")),
    )
    _CACHE["last_results"] = res

    out = np.empty((B, N, D), dtype=np.float32)
    for b in range(B):
        offset = np.zeros((D,), dtype=np.float32)
        for c in range(chunks_per_b):
            core_out = res.results[b * chunks_per_b + c]["out"].astype(np.float32)
            out[b, c * CHUNK:(c + 1) * CHUNK] = core_out + offset[None, :]
            offset = out[b, (c + 1) * CHUNK - 1].copy()

    if np.any(b1 != 0.0):
        # the kernel ignores b1 (it cancels in all diffs except row 0);
        # swap row 0's increment for the exact fp32 one including b1.
        W1q_f = W1q.astype(np.float32)
        for b in range(B):
            d0_q = np.clip(dx[b, 0], -FP8_MAX, FP8_MAX).astype(FP8).astype(np.float32)
            m_kern = np.abs(d0_q @ W1q_f).reshape(K, D).sum(axis=0) / WSCALE
            v_true = x[b, 0] @ W1 + b1
            m_true = (np.abs(v_true.reshape(K, D)) * scale[:, None]).sum(axis=0)
            out[b] += (m_true - m_kern)[None, :]

    return out


# revision 48
# speedup vs baseline: 1.0024x; 1.0024x over previous
"""Trainium2 Bass kernel for nn_BulkSpaceGenerator.

Computes, for boundary_tokens x (B, N, D), W1 (D, K*D), b1 (K*D,):
    bulk   = x @ W1 + b1                    -> (B, N, K, D)
    inc    = |delta_n bulk| * (ads/z_k)     (delta along sequence, first row = bulk[0])
    out    = cumsum_n(inc).mean(k)          -> (B, N, D)

Key algebraic restructuring:
  - mean over k commutes with the cumsum, so out = cumsum_n(mean_k(warp_k*|delta|)).
  - delta_n bulk = (delta_n x) @ W1 (bias cancels for n>0), so we matmul the
    *differenced* input once instead of materializing bulk.
  - warp_k/K is positive, so it folds into W1's columns: |dx @ (W1*s_k)| = s_k|dx @ W1|.

The big matmul runs in fp8-e4m3 with perf_mode=DoubleRow (2 fp8 weights per PE
cell, contraction 256 per instruction) for ~1.9x TensorE throughput over bf16.
Weights carry a global x64 scale (keeps fp8 values out of the subnormal
range); since everything after the |.| is linear, the 1/64 descale folds into
the final PSUM->SBUF output copy.

Sharding: 8 shards over (B=2) x (4 sequence chunks of 1024 tokens). Each core
computes its chunk's per-token increments m = sum_k |dxT.T @ W1s|_k and the
local cumsum on-device; the host adds the (tiny) cross-chunk prefix offsets.

Device layout per core (tokens on PSUM partitions, so the local cumsum is a
matmul with a triangular ones matrix and output rows DMA out contiguously):
  dxt  (128, 8, 8, 128) fp8  [p, tb, cb, t] = dx_chunk[tb*128+t, cb*128+p]
  w<k> (128, 8, 1024) fp8    [p, cb, d]     = (W1 * 64*s_k)[cb*128+p, k*1024+d]
  out  (1024, 1024) bf16     local cumsum of m over the chunk (scaled 1/64)

The k-sum accumulates in bf16 (ACT abs-evacuates PSUM -> bf16, DVE adds at 2x
rate); the cumsum is tri@acc[tb] + ones@S_bf[tb] per 128-token block. S_bf is
a per-partition bf16 running sum of whole blocks -- the ones-matmul reduces it
over partitions in f32 PSUM, so each stored entry stays ~128x below the true
prefix and the bf16 chain error is negligible. Emission lags its block's
evacuation by one block so the cumsum matmuls never stall the PE.
"""

import os
import sys
import types
import numpy as np
import ml_dtypes

D = 1024
K = 10
B = 2
N = 4096
ADS_RADIUS = 1.0
NCORES = 8
CHUNK = 1024            # tokens per core
KD = K * D
CB = 8                  # contraction blocks (D / 128)
TBLK = 8                # token blocks per chunk (CHUNK / 128)
# kd columns per weight group: one k slice per group
GROUP_COLS = [D] * K
GROUP_LO = [sum(GROUP_COLS[:g]) for g in range(len(GROUP_COLS))]
NGROUPS = len(GROUP_COLS)

BF16 = ml_dtypes.bfloat16
FP8 = ml_dtypes.float8_e4m3   # TRN FP8_EXP4: max normal +-240
FP8_MAX = 240.0
WSCALE = 64.0                 # global fp8 weight scale, descaled at output

_CACHE = {}


def _install_ntff_hook():
    """Best-effort: register the axon NTFF profiling hook so BASS_TRACE=1 works.

    The agent image's antenv package lacks axon_hooks; inject a shim module and
    wire it to the ctypes-based hook from trn_agent_boot. Harmless if anything
    is missing -- tracing is simply skipped.
    """
    try:
        import antenv
        if "antenv.axon_hooks" in sys.modules:
            return
        hooks = []
        mod = types.ModuleType("antenv.axon_hooks")
        mod.set_axon_ntff_profile_hook = hooks.append
        mod.get_axon_ntff_profile_hook = lambda: (hooks[-1] if hooks else None)
        sys.modules["antenv.axon_hooks"] = mod
        antenv.axon_hooks = mod
        from trn_agent_boot.trn_boot import _ntff_profile_via_ctypes
        h = _ntff_profile_via_ctypes("/opt/axon/libaxon_pjrt.so")
        if h is not None:
            mod.set_axon_ntff_profile_hook(h)
    except Exception:
        pass


def _build():
    from concourse import bacc
    import concourse.mybir as mybir
    import concourse.tile as tile

    fp32 = mybir.dt.float32
    bf16 = mybir.dt.bfloat16
    fp8 = mybir.dt.float8e4
    ADD = mybir.AluOpType.add
    ABS = mybir.ActivationFunctionType.Abs
    DR = mybir.MatmulPerfMode.DoubleRow

    nc = bacc.Bacc()
    dxt = nc.declare_dram_parameter("dxt", [128, TBLK, CB, 128], fp8, isOutput=False)
    wg = [
        nc.declare_dram_parameter(f"w{g}", [128, CB, GROUP_COLS[g]], fp8, isOutput=False)
        for g in range(NGROUPS)
    ]
    tri = nc.declare_dram_parameter("tri", [128, 128], bf16, isOutput=False)
    ones = nc.declare_dram_parameter("ones", [128, 128], bf16, isOutput=False)
    out = nc.declare_dram_parameter("out", [CHUNK, D], bf16, isOutput=True)

    with tile.TileContext(nc) as tc:
        with (
            tc.tile_pool(name="const", bufs=1) as cpool,
            tc.tile_pool(name="dx", bufs=1) as dxpool,
            tc.tile_pool(name="w", bufs=2) as wpool,
            tc.tile_pool(name="acc", bufs=1) as accpool,
            tc.tile_pool(name="tmp", bufs=3) as tpool,
            tc.tile_pool(name="outs", bufs=3) as opool,
            tc.tile_pool(name="ps", bufs=2, space="PSUM") as ppool,
            tc.tile_pool(name="pc", bufs=4, space="PSUM") as pcpool,
        ):
            tri_sb = cpool.tile([128, 128], bf16, tag="tri")
            ones_sb = cpool.tile([128, 128], bf16, tag="ones")
            dx_sb = dxpool.tile([128, TBLK, CB, 128], fp8, tag="dxt")

            # small constants ride the ACT HWDGE ring so they never queue
            # ahead of the weight/dx stream on the sync ring
            nc.scalar.dma_start(out=tri_sb[:], in_=tri[:])
            nc.scalar.dma_start(out=ones_sb[:], in_=ones[:])

            acc = accpool.tile([128, TBLK, D], bf16, tag="acc")
            # S_bf[tb] = per-partition bf16 running sum of acc[0..tb-1]; the
            # ones-matmul reduces over partitions, so each entry stays ~128x
            # below the true prefix and the bf16 chain error is negligible
            S_bf = accpool.tile([128, TBLK, D], bf16, tag="Sbf")

            def emit(tb):
                # out rows of block tb = tri @ acc[tb] + ones @ S_bf[tb]
                ot = opool.tile([128, D], bf16, tag="ot", name="ot")
                for h in range(2):
                    hs = slice(h * 512, (h + 1) * 512)
                    pc = pcpool.tile([128, 512], fp32, tag="pc", name="pc")
                    nc.tensor.matmul(
                        pc[:], lhsT=tri_sb[:], rhs=acc[:, tb, hs],
                        start=True, stop=(tb == 0),
                    )
                    if tb > 0:
                        nc.tensor.matmul(
                            pc[:], lhsT=ones_sb[:], rhs=S_bf[:, tb, hs],
                            start=False, stop=True,
                        )
                    # descale + PSUM->SBUF on two engines in parallel; each
                    # half DMAs out as it lands, on the (long idle) sync ring
                    if h == 0:
                        nc.scalar.mul(ot[:, hs], pc[:], 1.0 / WSCALE)
                    else:
                        nc.vector.tensor_scalar_mul(ot[:, hs], pc[:], 1.0 / WSCALE)
                    nc.sync.dma_start(
                        out=out[tb * 128:(tb + 1) * 128, hs], in_=ot[:, hs]
                    )

            def dma_dx(tb):
                nc.sync.dma_start(out=dx_sb[:, tb, :, :], in_=dxt[:, tb, :, :])

            for g in range(NGROUPS):
                gcols = GROUP_COLS[g]
                jt = gcols // 512
                wt = wpool.tile([128, CB, gcols], fp8, tag="wt", name="wt")

                def dma_w(p, g=g, wt=wt):
                    nc.sync.dma_start(
                        out=wt[:, 2 * p:2 * p + 2, :], in_=wg[g][:, 2 * p:2 * p + 2, :]
                    )

                if g == 0:
                    # the first matmul needs only dx slab 0 + weight pair 0;
                    # land those first. Pairs 2/3 ride the scalar HWDGE ring
                    # in parallel so block 0's full contraction arrives in
                    # time, while the dx slabs stream on the sync ring.
                    dma_dx(0)
                    dma_w(0)
                    dma_w(1)
                    nc.scalar.dma_start(out=wt[:, 4:6, :], in_=wg[0][:, 4:6, :])
                    nc.scalar.dma_start(out=wt[:, 6:8, :], in_=wg[0][:, 6:8, :])
                    for tb in range(1, TBLK):
                        dma_dx(tb)
                else:
                    for p in range(CB // 2):
                        dma_w(p)

                last = g == NGROUPS - 1
                # PSUM accumulation order is free; for group 0 follow the
                # order the weight pairs actually arrive on the two rings
                # (pair 1 queues behind the dx slabs on the sync ring)
                cbp_order = [0, 2, 3, 1] if g == 0 else [0, 1, 2, 3]
                base = GROUP_LO[g] // 512          # first 512-col kd tile
                for tb in range(TBLK):
                    ps = ppool.tile([128, gcols], fp32, tag="ps", name="ps")
                    for ci, cbp in enumerate(cbp_order):
                        lhsT = dx_sb[:, tb, 2 * cbp:2 * cbp + 2, :]
                        for j in range(jt):
                            mi = nc.tensor.matmul(
                                ps[:, j * 512:(j + 1) * 512],
                                lhsT=lhsT,
                                rhs=wt[:, 2 * cbp:2 * cbp + 2, j * 512:(j + 1) * 512],
                                start=(ci == 0),
                                stop=(ci == CB // 2 - 1),
                                perf_mode=DR,
                            )
                            if j > 0:
                                # same stationary as the previous matmul (the
                                # two j tiles share lhsT and are adjacent in
                                # the PE stream): skip the redundant reload
                                mi.ins.ldweights = False
                    # evacuate |psum| and accumulate into acc[tb]; kd tile
                    # base+j lands at acc column ((base+j)%2)*512, merging
                    # tile pairs whose destinations are contiguous
                    if last and tb == TBLK - 1:
                        # split the final evacuation into halves so the last
                        # block's emission matmuls start on half 0 while half
                        # 1 is still coming out of PSUM (shortens the tail)
                        tmp = tpool.tile([128, gcols], bf16, tag="tmp", name="tmp")
                        for h in range(2):
                            hs = slice(h * 512, (h + 1) * 512)
                            nc.scalar.activation(tmp[:, hs], ps[:, hs], ABS)
                            nc.vector.tensor_tensor(
                                acc[:, tb, hs], acc[:, tb, hs], tmp[:, hs], ADD,
                            )
                    else:
                        # kd tiles 0/1 are the first touch of their acc half:
                        # ACT writes |psum| straight in, no add needed
                        tmp = None
                        if base + jt > 2:
                            tmp = tpool.tile([128, gcols], bf16, tag="tmp", name="tmp")
                            nc.scalar.activation(tmp[:], ps[:], ABS)
                        j = 0
                        while j < jt:
                            w = 2 if ((base + j) % 2 == 0 and j + 1 < jt) else 1
                            doff = ((base + j) % 2) * 512
                            dest = acc[:, tb, doff:doff + 512 * w]
                            if base + j < 2:
                                nc.scalar.activation(
                                    dest, ps[:, j * 512:(j + w) * 512], ABS
                                )
                            else:
                                nc.vector.tensor_tensor(
                                    dest, dest, tmp[:, j * 512:(j + w) * 512], ADD,
                                )
                            j += w
                    if last:
                        if tb >= 1:
                            if tb == 1:
                                nc.vector.tensor_copy(S_bf[:, 1, :], acc[:, 0, :])
                            else:
                                nc.vector.tensor_tensor(
                                    S_bf[:, tb, :], S_bf[:, tb - 1, :],
                                    acc[:, tb - 1, :], ADD,
                                )
                            # emission lags one block so its matmuls never
                            # wait on this block's evacuation chain
                            emit(tb - 1)
            emit(TBLK - 1)

    nc.compile()
    return nc


def _get_nc():
    if "nc" not in _CACHE:
        _CACHE["nc"] = _build()
    return _CACHE["nc"]


def kernel(boundary_tokens: np.ndarray, W1: np.ndarray, b1: np.ndarray) -> np.ndarray:
    from concourse.bass_utils import run_bass_kernel_spmd

    _install_ntff_hook()

    x = np.asarray(boundary_tokens, dtype=np.float32)
    W1 = np.asarray(W1, dtype=np.float32)
    b1 = np.asarray(b1, dtype=np.float32)
    assert x.shape == (B, N, D) and W1.shape == (D, KD)

    # host prep: difference along the sequence, fold warp/K scaling into W1
    dx = np.empty_like(x)
    dx[:, 0] = x[:, 0]
    dx[:, 1:] = x[:, 1:] - x[:, :-1]

    scale = (1.0 / (np.arange(K, dtype=np.float32) + 1.0))  # warp_k / K = 1/(k+1)
    W1s = (W1.reshape(D, K, D) * (WSCALE * scale)[None, :, None]).reshape(D, KD)
    W1q = np.clip(W1s, -FP8_MAX, FP8_MAX).astype(FP8)
    # [p, cb, col] per column group, each contiguous so weight DMAs get >=2KB rows
    w_r = W1q.reshape(CB, 128, KD)
    w_groups = [
        np.ascontiguousarray(
            w_r[:, :, GROUP_LO[g]:GROUP_LO[g] + GROUP_COLS[g]].transpose(1, 0, 2)
        )
        for g in range(NGROUPS)
    ]

    idx = np.arange(128)
    tri = (idx[:, None] <= idx[None, :]).astype(BF16)   # tri[s,t]=1 iff s<=t
    ones = np.ones((128, 128), dtype=BF16)

    chunks_per_b = N // CHUNK
    in_maps = []
    for core in range(NCORES):
        b, c = divmod(core, chunks_per_b)
        dxc = dx[b, c * CHUNK:(c + 1) * CHUNK]          # (CHUNK, D)
        dxq = np.clip(dxc, -FP8_MAX, FP8_MAX).astype(FP8)
        # [p, tb, cb, t]: per token-block slab, contiguous 1KB rows
        dxt = np.ascontiguousarray(
            dxq.T.reshape(CB, 128, TBLK, 128).transpose(1, 2, 0, 3)
        )
        im = {"dxt": dxt, "tri": tri, "ones": ones}
        for g in range(NGROUPS):
            im[f"w{g}"] = w_groups[g]
        in_maps.append(im)

    res = run_bass_kernel_spmd(
        _get_nc(), in_maps, list(range(NCORES)),
        trace=bool(os.environ.get("BASS_TRACE")),
    )
    _CACHE["last_results"] = res

    out = np.empty((B, N, D), dtype=np.float32)
    for b in range(B):
        offset = np.zeros((D,), dtype=np.float32)
        for c in range(chunks_per_b):
            core_out = res.results[b * chunks_per_b + c]["out"].astype(np.float32)
            out[b, c * CHUNK:(c + 1) * CHUNK] = core_out + offset[None, :]
            offset = out[b, (c + 1) * CHUNK - 1].copy()

    if np.any(b1 != 0.0):
        # the kernel ignores b1 (it cancels in all diffs except row 0);
        # swap row 0's increment for the exact fp32 one including b1.
        W1q_f = W1q.astype(np.float32)
        for b in range(B):
            d0_q = np.clip(dx[b, 0], -FP8_MAX, FP8_MAX).astype(FP8).astype(np.float32)
            m_kern = np.abs(d0_q @ W1q_f).reshape(K, D).sum(axis=0) / WSCALE
            v_true = x[b, 0] @ W1 + b1
            m_true = (np.abs(v_true.reshape(K, D)) * scale[:, None]).sum(axis=0)
            out[b] += (m_true - m_kern)[None, :]

    return out


# revision 49
# speedup vs baseline: 1.0104x; 1.0080x over previous
"""Trainium2 Bass kernel for nn_BulkSpaceGenerator.

Computes, for boundary_tokens x (B, N, D), W1 (D, K*D), b1 (K*D,):
    bulk   = x @ W1 + b1                    -> (B, N, K, D)
    inc    = |delta_n bulk| * (ads/z_k)     (delta along sequence, first row = bulk[0])
    out    = cumsum_n(inc).mean(k)          -> (B, N, D)

Key algebraic restructuring:
  - mean over k commutes with the cumsum, so out = cumsum_n(mean_k(warp_k*|delta|)).
  - delta_n bulk = (delta_n x) @ W1 (bias cancels for n>0), so we matmul the
    *differenced* input once instead of materializing bulk.
  - warp_k/K is positive, so it folds into W1's columns: |dx @ (W1*s_k)| = s_k|dx @ W1|.

The big matmul runs in fp8-e4m3 with perf_mode=DoubleRow (2 fp8 weights per PE
cell, contraction 256 per instruction) for ~1.9x TensorE throughput over bf16.
Weights carry a global x64 scale (keeps fp8 values out of the subnormal
range); since everything after the |.| is linear, the 1/64 descale folds into
the final PSUM->SBUF output copy.

Sharding: 8 shards over (B=2) x (4 sequence chunks of 1024 tokens). Each core
computes its chunk's per-token increments m = sum_k |dxT.T @ W1s|_k and the
local cumsum on-device; the host adds the (tiny) cross-chunk prefix offsets.

Device layout per core (tokens on PSUM partitions, so the local cumsum is a
matmul with a triangular ones matrix and output rows DMA out contiguously):
  dxt  (128, 8, 8, 128) fp8  [p, tb, cb, t] = dx_chunk[tb*128+t, cb*128+p]
  w<k> (128, 8, 1024) fp8    [p, cb, d]     = (W1 * 64*s_k)[cb*128+p, k*1024+d]
  out  (1024, 1024) bf16     local cumsum of m over the chunk (scaled 1/64)

The k-sum accumulates in bf16 (ACT abs-evacuates PSUM -> bf16, DVE adds at 2x
rate); the cumsum is tri@acc[tb] + ones@S_bf[tb] per 128-token block. S_bf is
a per-partition bf16 running sum of whole blocks -- the ones-matmul reduces it
over partitions in f32 PSUM, so each stored entry stays ~128x below the true
prefix and the bf16 chain error is negligible. Emission lags its block's
evacuation by one block so the cumsum matmuls never stall the PE.
"""

import os
import sys
import types
import numpy as np
import ml_dtypes

D = 1024
K = 10
B = 2
N = 4096
ADS_RADIUS = 1.0
NCORES = 8
CHUNK = 1024            # tokens per core
KD = K * D
CB = 8                  # contraction blocks (D / 128)
TBLK = 8                # token blocks per chunk (CHUNK / 128)
# kd columns per weight group: one k slice per group
GROUP_COLS = [D] * K
GROUP_LO = [sum(GROUP_COLS[:g]) for g in range(len(GROUP_COLS))]
NGROUPS = len(GROUP_COLS)

BF16 = ml_dtypes.bfloat16
FP8 = ml_dtypes.float8_e4m3   # TRN FP8_EXP4: max normal +-240
FP8_MAX = 240.0
WSCALE = 64.0                 # global fp8 weight scale, descaled at output

_CACHE = {}


def _install_ntff_hook():
    """Best-effort: register the axon NTFF profiling hook so BASS_TRACE=1 works.

    The agent image's antenv package lacks axon_hooks; inject a shim module and
    wire it to the ctypes-based hook from trn_agent_boot. Harmless if anything
    is missing -- tracing is simply skipped.
    """
    try:
        import antenv
        if "antenv.axon_hooks" in sys.modules:
            return
        hooks = []
        mod = types.ModuleType("antenv.axon_hooks")
        mod.set_axon_ntff_profile_hook = hooks.append
        mod.get_axon_ntff_profile_hook = lambda: (hooks[-1] if hooks else None)
        sys.modules["antenv.axon_hooks"] = mod
        antenv.axon_hooks = mod
        from trn_agent_boot.trn_boot import _ntff_profile_via_ctypes
        h = _ntff_profile_via_ctypes("/opt/axon/libaxon_pjrt.so")
        if h is not None:
            mod.set_axon_ntff_profile_hook(h)
    except Exception:
        pass


def _build():
    from concourse import bacc
    import concourse.mybir as mybir
    import concourse.tile as tile

    fp32 = mybir.dt.float32
    bf16 = mybir.dt.bfloat16
    fp8 = mybir.dt.float8e4
    ADD = mybir.AluOpType.add
    ABS = mybir.ActivationFunctionType.Abs
    DR = mybir.MatmulPerfMode.DoubleRow

    nc = bacc.Bacc()
    dxt = nc.declare_dram_parameter("dxt", [128, TBLK, CB, 128], fp8, isOutput=False)
    wg = [
        nc.declare_dram_parameter(f"w{g}", [128, CB, GROUP_COLS[g]], fp8, isOutput=False)
        for g in range(NGROUPS)
    ]
    tri = nc.declare_dram_parameter("tri", [128, 128], bf16, isOutput=False)
    ones = nc.declare_dram_parameter("ones", [128, 128], bf16, isOutput=False)
    out = nc.declare_dram_parameter("out", [CHUNK, D], bf16, isOutput=True)

    with tile.TileContext(nc) as tc:
        with (
            tc.tile_pool(name="const", bufs=1) as cpool,
            tc.tile_pool(name="dx", bufs=1) as dxpool,
            tc.tile_pool(name="w", bufs=2) as wpool,
            tc.tile_pool(name="acc", bufs=1) as accpool,
            tc.tile_pool(name="tmp", bufs=3) as tpool,
            tc.tile_pool(name="outs", bufs=3) as opool,
            tc.tile_pool(name="ps", bufs=2, space="PSUM") as ppool,
            tc.tile_pool(name="pc", bufs=4, space="PSUM") as pcpool,
        ):
            tri_sb = cpool.tile([128, 128], bf16, tag="tri")
            ones_sb = cpool.tile([128, 128], bf16, tag="ones")
            dx_sb = dxpool.tile([128, TBLK, CB, 128], fp8, tag="dxt")

            # small constants ride the ACT HWDGE ring so they never queue
            # ahead of the weight/dx stream on the sync ring
            nc.scalar.dma_start(out=tri_sb[:], in_=tri[:])
            nc.scalar.dma_start(out=ones_sb[:], in_=ones[:])

            acc = accpool.tile([128, TBLK, D], bf16, tag="acc")
            # S_bf[tb] = per-partition bf16 running sum of acc[0..tb-1]; the
            # ones-matmul reduces over partitions, so each entry stays ~128x
            # below the true prefix and the bf16 chain error is negligible
            S_bf = accpool.tile([128, TBLK, D], bf16, tag="Sbf")

            def emit(tb):
                # out rows of block tb = tri @ acc[tb] + ones @ S_bf[tb]
                ot = opool.tile([128, D], bf16, tag="ot", name="ot")
                for h in range(2):
                    hs = slice(h * 512, (h + 1) * 512)
                    pc = pcpool.tile([128, 512], fp32, tag="pc", name="pc")
                    nc.tensor.matmul(
                        pc[:], lhsT=tri_sb[:], rhs=acc[:, tb, hs],
                        start=True, stop=(tb == 0),
                    )
                    if tb > 0:
                        nc.tensor.matmul(
                            pc[:], lhsT=ones_sb[:], rhs=S_bf[:, tb, hs],
                            start=False, stop=True,
                        )
                    # descale + PSUM->SBUF on two engines in parallel; each
                    # half DMAs out as it lands, on the (long idle) sync ring
                    if h == 0:
                        nc.scalar.mul(ot[:, hs], pc[:], 1.0 / WSCALE)
                    else:
                        nc.vector.tensor_scalar_mul(ot[:, hs], pc[:], 1.0 / WSCALE)
                    nc.sync.dma_start(
                        out=out[tb * 128:(tb + 1) * 128, hs], in_=ot[:, hs]
                    )

            def dma_dx(tb):
                nc.sync.dma_start(out=dx_sb[:, tb, :, :], in_=dxt[:, tb, :, :])

            for g in range(NGROUPS):
                gcols = GROUP_COLS[g]
                jt = gcols // 512
                wt = wpool.tile([128, CB, gcols], fp8, tag="wt", name="wt")

                def dma_w(p, g=g, wt=wt):
                    nc.sync.dma_start(
                        out=wt[:, 2 * p:2 * p + 2, :], in_=wg[g][:, 2 * p:2 * p + 2, :]
                    )

                if g == 0:
                    # the first matmul needs only dx slab 0 + weight pair 0;
                    # land those first. Pairs 2/3 ride the scalar HWDGE ring
                    # in parallel so block 0's full contraction arrives in
                    # time, while the dx slabs stream on the sync ring.
                    dma_dx(0)
                    dma_w(0)
                    dma_w(1)
                    nc.scalar.dma_start(out=wt[:, 4:6, :], in_=wg[0][:, 4:6, :])
                    nc.scalar.dma_start(out=wt[:, 6:8, :], in_=wg[0][:, 6:8, :])
                    for tb in range(1, TBLK):
                        dma_dx(tb)
                else:
                    for p in range(CB // 2):
                        dma_w(p)

                last = g == NGROUPS - 1
                # PSUM accumulation order is free; for group 0 follow the
                # order the weight pairs actually arrive on the two rings
                # (pair 1 queues behind the dx slabs on the sync ring)
                cbp_order = [0, 2, 3, 1] if g == 0 else [0, 1, 2, 3]
                base = GROUP_LO[g] // 512          # first 512-col kd tile
                for tb in range(TBLK):
                    ps = ppool.tile([128, gcols], fp32, tag="ps", name="ps")
                    for ci, cbp in enumerate(cbp_order):
                        lhsT = dx_sb[:, tb, 2 * cbp:2 * cbp + 2, :]
                        for j in range(jt):
                            nc.tensor.matmul(
                                ps[:, j * 512:(j + 1) * 512],
                                lhsT=lhsT,
                                rhs=wt[:, 2 * cbp:2 * cbp + 2, j * 512:(j + 1) * 512],
                                start=(ci == 0),
                                stop=(ci == CB // 2 - 1),
                                perf_mode=DR,
                            )
                    # evacuate |psum| and accumulate into acc[tb]; kd tile
                    # base+j lands at acc column ((base+j)%2)*512, merging
                    # tile pairs whose destinations are contiguous
                    if last and tb == TBLK - 1:
                        # split the final evacuation into halves so the last
                        # block's emission matmuls start on half 0 while half
                        # 1 is still coming out of PSUM (shortens the tail)
                        tmp = tpool.tile([128, gcols], bf16, tag="tmp", name="tmp")
                        for h in range(2):
                            hs = slice(h * 512, (h + 1) * 512)
                            nc.scalar.activation(tmp[:, hs], ps[:, hs], ABS)
                            nc.vector.tensor_tensor(
                                acc[:, tb, hs], acc[:, tb, hs], tmp[:, hs], ADD,
                            )
                    else:
                        # kd tiles 0/1 are the first touch of their acc half:
                        # ACT writes |psum| straight in, no add needed
                        tmp = None
                        if base + jt > 2:
                            tmp = tpool.tile([128, gcols], bf16, tag="tmp", name="tmp")
                            nc.scalar.activation(tmp[:], ps[:], ABS)
                        j = 0
                        while j < jt:
                            w = 2 if ((base + j) % 2 == 0 and j + 1 < jt) else 1
                            doff = ((base + j) % 2) * 512
                            dest = acc[:, tb, doff:doff + 512 * w]
                            if base + j < 2:
                                nc.scalar.activation(
                                    dest, ps[:, j * 512:(j + w) * 512], ABS
                                )
                            else:
                                nc.vector.tensor_tensor(
                                    dest, dest, tmp[:, j * 512:(j + w) * 512], ADD,
                                )
                            j += w
                    if last:
                        if tb >= 1:
                            if tb == 1:
                                nc.vector.tensor_copy(S_bf[:, 1, :], acc[:, 0, :])
                            else:
                                nc.vector.tensor_tensor(
                                    S_bf[:, tb, :], S_bf[:, tb - 1, :],
                                    acc[:, tb - 1, :], ADD,
                                )
                            # emission lags one block so its matmuls never
                            # wait on this block's evacuation chain
                            emit(tb - 1)
            emit(TBLK - 1)

    nc.compile()
    return nc


def _get_nc():
    if "nc" not in _CACHE:
        _CACHE["nc"] = _build()
    return _CACHE["nc"]


def kernel(boundary_tokens: np.ndarray, W1: np.ndarray, b1: np.ndarray) -> np.ndarray:
    from concourse.bass_utils import run_bass_kernel_spmd

    _install_ntff_hook()

    x = np.asarray(boundary_tokens, dtype=np.float32)
    W1 = np.asarray(W1, dtype=np.float32)
    b1 = np.asarray(b1, dtype=np.float32)
    assert x.shape == (B, N, D) and W1.shape == (D, KD)

    # host prep: difference along the sequence, fold warp/K scaling into W1
    dx = np.empty_like(x)
    dx[:, 0] = x[:, 0]
    dx[:, 1:] = x[:, 1:] - x[:, :-1]

    scale = (1.0 / (np.arange(K, dtype=np.float32) + 1.0))  # warp_k / K = 1/(k+1)
    W1s = (W1.reshape(D, K, D) * (WSCALE * scale)[None, :, None]).reshape(D, KD)
    W1q = np.clip(W1s, -FP8_MAX, FP8_MAX).astype(FP8)
    # [p, cb, col] per column group, each contiguous so weight DMAs get >=2KB rows
    w_r = W1q.reshape(CB, 128, KD)
    w_groups = [
        np.ascontiguousarray(
            w_r[:, :, GROUP_LO[g]:GROUP_LO[g] + GROUP_COLS[g]].transpose(1, 0, 2)
        )
        for g in range(NGROUPS)
    ]

    idx = np.arange(128)
    tri = (idx[:, None] <= idx[None, :]).astype(BF16)   # tri[s,t]=1 iff s<=t
    ones = np.ones((128, 128), dtype=BF16)

    chunks_per_b = N // CHUNK
    in_maps = []
    for core in range(NCORES):
        b, c = divmod(core, chunks_per_b)
        dxc = dx[b, c * CHUNK:(c + 1) * CHUNK]          # (CHUNK, D)
        dxq = np.clip(dxc, -FP8_MAX, FP8_MAX).astype(FP8)
        # [p, tb, cb, t]: per token-block slab, contiguous 1KB rows
        dxt = np.ascontiguousarray(
            dxq.T.reshape(CB, 128, TBLK, 128).transpose(1, 2, 0, 3)
        )
        im = {"dxt": dxt, "tri": tri, "ones": ones}
        for g in range(NGROUPS):
            im[f"w{g}"] = w_groups[g]
        in_maps.append(im)

    res = run_bass_kernel_spmd(
        _get_nc(), in_maps, list(range(NCORES)),
        trace=bool(os.environ.get("BASS_TRACE")),
    )
    _CACHE["last_results"] = res

    out = np.empty((B, N, D), dtype=np.float32)
    for b in range(B):
        offset = np.zeros((D,), dtype=np.float32)
        for c in range(chunks_per_b):
            core_out = res.results[b * chunks_per_b + c]["out"].astype(np.float32)
            out[b, c * CHUNK:(c + 1) * CHUNK] = core_out + offset[None, :]
            offset = out[b, (c + 1) * CHUNK - 1].copy()

    if np.any(b1 != 0.0):
        # the kernel ignores b1 (it cancels in all diffs except row 0);
        # swap row 0's increment for the exact fp32 one including b1.
        W1q_f = W1q.astype(np.float32)
        for b in range(B):
            d0_q = np.clip(dx[b, 0], -FP8_MAX, FP8_MAX).astype(FP8).astype(np.float32)
            m_kern = np.abs(d0_q @ W1q_f).reshape(K, D).sum(axis=0) / WSCALE
            v_true = x[b, 0] @ W1 + b1
            m_true = (np.abs(v_true.reshape(K, D)) * scale[:, None]).sum(axis=0)
            out[b] += (m_true - m_kern)[None, :]

    return out
